# revision 13
# baseline (speedup 1.0000x reference)
"""Trainium2 Bass kernel: Chebyshev graph convolution.

Computes  out = sum_k A_k @ (x @ W_k) + bias  where A_k are sparse COO
matrices sharing one sparsity pattern (edge_row/edge_col) with per-degree
values adj_vals[k].

Restructured as:
    G      = x[edge_col]                       (host gather, once per edge)
    Y_k    = segment_sum(adj_vals[k] * G)      (grouped one-hot matmuls on PE)
    out    = sum_k Y_k @ W_k + bias            (dense, fused into the scatter)

Sharding: destination-node range across 8 cores (6250 rows each), zero
cross-core communication.

Device pipeline per superunit (4 units; one unit = 512 sorted+padded edge
slots = 32 groups of 16, each group summing into one dest row;
j = 32*cc + 8*k + g):
    G_cc[e, fi]      = x[col[e], :]              (streamed dense DMA)
    P4w[e, j]        = pattern[e,g] * a_k[e]     (pre-baked on HOST, streamed)
    Yp[fi, j]       += G_cc^T @ P4w_cc           (M1: 16 matmuls, bf16 PSUM)
    yp_sb            = copy(Yp)                  (Pool engine)
    ZT[fo, j]        = W_k^T @ Yp[:, kslice]     (M2: 16 matmuls, bf16 PSUM)
    zT_sb            = copy(ZT)                  (ACT engine)
    Zp_u             = ZT_u^T                    (PE transpose, per unit)
    zp_sb            = copy(Zp)                  (DVE engine)
    P2_u[j, d]       = (iota_d == skey[j])       (DVE tensor_scalar, per unit)
    acc[d, fo]      += P2_u^T @ Zp_u             (M3, f32 PSUM, 4 blocks per
                                                  acc tile, bias-seeded)
    out[4 blocks]    = copy(acc) ; DMA out       (Pool copy, one DMA)
"""

import numpy as np

N_NODES = 50000
N_EDGES = 800000
F = 128
K = 4
N_CORES = 8
RPC = N_NODES // N_CORES      # rows per core
NBLK = (RPC + 127) // 128     # dest blocks per core (49)
GS = 16                       # edges per group (one dest row per group)
GPU = 32                      # groups per unit
EPU = GS * GPU                # edges per unit (512)
SU = 4                        # units per superunit (PSUM bank = [128, 512])
RND = 8                       # units per staging round (2 superunits)
BB = 4                        # blocks per output batch

F32 = np.float32
I32 = np.int32

USE_FP8_G = False             # fp8e4m3 for the gathered-feature stream


# ----------------------------------------------------------------------------
# Host-side preprocessing: shard + sort + pad the edge list, build payloads.
# ----------------------------------------------------------------------------

def _preprocess(adj_vals, edge_row, edge_col):
    """Build per-core payload arrays and the shared static schedule.

    Returns (U_bs, payloads): U_bs[b] = units of block b (shared by all
    cores; padded so sum(U_bs) % RND == 0); payloads[c] = dict(cols_unit
    [U*512] int32, a4 [U*512, K] f32, skey_j [U, 128] f32).
    """
    import heapq

    er = np.asarray(edge_row, dtype=np.int64)
    ec = np.asarray(edge_col, dtype=np.int64)
    adj = np.asarray(adj_vals, dtype=F32)            # [K, E]

    # Balanced dest-row assignment: LPT-pack rows into the 392 (core, block)
    # bins by group count, so every block needs the same number of units on
    # every core (U_bs is a max over cores).
    counts_all = np.bincount(er, minlength=N_NODES)
    g_all = -(-counts_all // GS)
    nbins = N_CORES * NBLK
    cap = np.full(nbins, 128, dtype=np.int64)
    cap[NBLK - 1::NBLK] = RPC - 128 * (NBLK - 1)
    fill = np.zeros(nbins, dtype=np.int64)
    assign_loc = np.empty(N_NODES, dtype=np.int64)   # core*RPC + local row
    heap = [(200 if i % NBLK == NBLK - 1 else 0, i) for i in range(nbins)]
    heapq.heapify(heap)
    for r in np.argsort(-g_all, kind="stable"):
        while True:
            load, b = heapq.heappop(heap)
            if fill[b] < cap[b]:
                break
        c_, blk = divmod(b, NBLK)
        assign_loc[r] = c_ * RPC + blk * 128 + fill[b]
        fill[b] += 1
        if fill[b] < cap[b]:
            heapq.heappush(heap, (load + int(g_all[r]), b))
    devpos_of_row = assign_loc

    er_loc = assign_loc[er]                          # device position per edge
    core = er_loc // RPC
    per_core = []
    gpb_all = np.zeros((N_CORES, NBLK), dtype=np.int64)
    for c in range(N_CORES):
        sel = np.nonzero(core == c)[0]
        rloc = (er_loc[sel] - c * RPC).astype(np.int64)
        order = np.argsort(rloc, kind="stable")
        eidx = sel[order]
        rs = rloc[order]
        counts = np.bincount(rs, minlength=RPC)       # edges per dest row
        gpr = -(-counts // GS)                        # groups per row
        gpr_pad = np.zeros(NBLK * 128, dtype=np.int64)
        gpr_pad[:RPC] = gpr
        gpb_all[c] = gpr_pad.reshape(NBLK, 128).sum(1)
        per_core.append((eidx, rs, counts, gpr))

    U_bs = np.maximum((-(-gpb_all // GPU)).max(axis=0), 1)  # units per block
    U = int(U_bs.sum())
    Upad = -(-U // RND) * RND
    U_bs[:Upad - U] += 1                   # spread pad units over first blocks
    U = Upad

    blk_grp_base = np.concatenate([[0], np.cumsum(U_bs * GPU)])[:-1]  # [NBLK]

    payloads = []
    for c in range(N_CORES):
        eidx, rs, counts, gpr = per_core[c]
        gpr_cum = np.concatenate([[0], np.cumsum(gpr)])  # [RPC+1]
        row_block = np.arange(RPC) // 128
        blk_first_row = row_block * 128
        grp_base_row = (blk_grp_base[row_block]
                        + gpr_cum[:RPC] - gpr_cum[blk_first_row])
        slot_base_row = grp_base_row * GS

        # scatter real edges into padded slots
        row_start = np.concatenate([[0], np.cumsum(counts)])  # [RPC+1]
        within = np.arange(len(rs)) - row_start[rs]
        pos = slot_base_row[rs] + within

        nslot = U * EPU
        cols_flat = np.zeros(nslot, dtype=I32)
        a4_flat = np.zeros((nslot, K), dtype=F32)
        cols_flat[pos] = ec[eidx].astype(I32)
        a4_flat[pos] = adj[:, eidx].T

        # dest row (mod 128) per group
        ngrp = U * GPU
        skey_grp = np.zeros(ngrp, dtype=F32)
        totg = int(gpr.sum())
        row_ids = np.repeat(np.arange(RPC), gpr)
        grp_within = np.arange(totg) - np.repeat(gpr_cum[:RPC], gpr)
        grp_pos = np.repeat(grp_base_row, gpr) + grp_within
        skey_grp[grp_pos] = (row_ids % 128).astype(F32)

        # per-unit skey over zp rows: j2 = 32*k + 8*cc + g  ->  group cc*8+g
        skey_j = np.tile(skey_grp.reshape(U, GPU), (1, K))

        payloads.append(dict(cols_unit=cols_flat, a4=a4_flat, skey_j=skey_j,
                             n_units=U))
    return U_bs.astype(np.int64), payloads, devpos_of_row


def _np_dt(dt_name):
    if dt_name == "bf16":
        import ml_dtypes
        return ml_dtypes.bfloat16
    if dt_name == "fp8":
        import ml_dtypes
        return ml_dtypes.float8_e4m3fn
    return F32


def _host_streams(x, weight, bias, payloads):
    """Per-core DMA streams, round-major so each DMA has one big contiguous
    descriptor per partition:
      g  [NRND, 128, RND*4*F]  (gdt)   unit ur chunk cc at col 512*ur+128*cc
      pw [NRND, 128, RND*129]  (bf16)  unit ur: 128 P4w cols + 1 skey col
    plus small consts."""
    bf = _np_dt("bf16")
    gdt = _np_dt("fp8" if USE_FP8_G else "bf16")
    xg = np.ascontiguousarray(np.asarray(x, dtype=F32).astype(gdt))
    w_packed = np.ascontiguousarray(
        np.asarray(weight, dtype=F32).transpose(1, 0, 2)
        .reshape(F, K * F).astype(bf))
    bias_row = np.asarray(bias, dtype=F32)[None, :].astype(bf).copy()
    iota = np.broadcast_to(
        np.arange(128, dtype=F32)[None, :], (128, 128)).astype(bf).copy()

    e_ar = np.arange(128)
    onehot_g = (e_ar[:, None] // GS == np.arange(8)[None, :]).astype(F32)

    in_maps = []
    for pl in payloads:
        U = pl["n_units"]
        NRND = U // RND
        st = np.empty((U, 128, 641), dtype=bf)
        # g block: [U, e, cc, fi]
        st[:, :, :512] = (xg[pl["cols_unit"]].astype(bf)
                          .reshape(U, 4, 128, F).transpose(0, 2, 1, 3)
                          .reshape(U, 128, 4 * F))
        # p4w: [U, e, 32cc+8k+g] = a4[slot(u,cc,e), k] * (e//16 == g)
        a4r = pl["a4"].reshape(U, 4, 128, K)             # [U, cc, e, k]
        st[:, :, 512:640] = np.einsum(
            "ucek,eg->ueckg", a4r, onehot_g).astype(bf).reshape(U, 128, 128)
        st[:, :, 640] = pl["skey_j"].astype(bf)
        st_rm = np.ascontiguousarray(
            st.reshape(NRND, RND, 128, 641).transpose(0, 2, 1, 3)
            .reshape(NRND, 128, RND * 641))
        in_maps.append({
            "st": st_rm,
            "w": w_packed, "bias_row": bias_row, "iota": iota,
        })
    return in_maps


# ----------------------------------------------------------------------------
# Numpy emulation of the device pipeline (layout validation / debugging).
# ----------------------------------------------------------------------------

def _emulate(x, weight, bias, U_bs, payloads):
    """Returns the DEVICE-ORDER output; index with devpos_of_row to compare."""
    in_maps = _host_streams(x, weight, bias, payloads)
    out = np.zeros((N_NODES, F), dtype=F32)
    blk_of_unit = np.repeat(np.arange(NBLK), U_bs)
    for c in range(N_CORES):
        im = in_maps[c]
        NRND = im["st"].shape[0]
        U = NRND * RND
        st = (im["st"].reshape(NRND, 128, RND, 641).transpose(0, 2, 1, 3)
              .reshape(U, 128, 641).astype(F32))
        g = st[:, :, :512].reshape(U, 128, 4, F)
        pw = st[:, :, 512:]
        w = im["w"].astype(F32)                       # [fi, K*F]
        iota = im["iota"].astype(F32)
        acc = np.zeros((NBLK, 128, F), dtype=F32)
        for u in range(U):
            b = int(blk_of_unit[u])
            yp = np.zeros((F, 128), dtype=F32)
            for cc in range(4):
                yp[:, 32 * cc:32 * cc + 32] = (
                    g[u, :, cc, :].T @ pw[u, :, 32 * cc:32 * cc + 32])
            zT = np.zeros((F, 128), dtype=F32)
            ypr = yp.reshape(F, 4, K, 8)
            for k in range(K):
                zT[:, 32 * k:32 * k + 32] = (
                    w[:, F * k:F * (k + 1)].T
                    @ ypr[:, :, k, :].reshape(F, 32))
            zp = zT.T
            p2 = (iota == pw[u, :, 128][:, None]).astype(F32)
            acc[b] += p2.T @ zp
        for b in range(NBLK):
            rows = min(128, RPC - 128 * b)
            out[c * RPC + 128 * b: c * RPC + 128 * b + rows] = (
                acc[b, :rows] + np.asarray(bias, F32)[None, :])
    return out


# ----------------------------------------------------------------------------
# Bass kernel builder.
# ----------------------------------------------------------------------------

def _build(U_bs):
    import concourse.bacc as bacc
    import concourse.mybir as mybir
    import concourse.tile as tile
    from concourse.masks import make_identity

    f32 = mybir.dt.float32
    bf16 = mybir.dt.bfloat16
    gdt = mybir.dt.float8e4 if USE_FP8_G else bf16
    # stream DMA split (engine, fraction of round); last gets remainder
    ST_SPLIT = (("sync", 0.43), ("scalar", 0.35), ("gpsimd", None))
    # odd-superunit zT copy: [0:c0] Pool, [c0:] DVE
    ZT_SPLIT = 128

    U = int(U_bs.sum())
    NRND = U // RND

    nc = bacc.Bacc("TRN2", target_bir_lowering=False, debug=False,
                   enable_asserts=False, num_devices=N_CORES)

    st_d = nc.dram_tensor("st", [NRND, 128, RND * 641], bf16,
                          kind="ExternalInput")
    w_d = nc.dram_tensor("w", [F, K * F], bf16, kind="ExternalInput")
    bias_d = nc.dram_tensor("bias_row", [1, F], bf16, kind="ExternalInput")
    iota_d = nc.dram_tensor("iota", [128, 128], bf16, kind="ExternalInput")
    out_d = nc.dram_tensor("out", [RPC, F], f32, kind="ExternalOutput")

    blk_of_unit = np.repeat(np.arange(NBLK), U_bs)
    ustart = np.concatenate([[0], np.cumsum(U_bs)])

    with tile.TileContext(nc) as tc:
        with (
            tc.tile_pool(name="const", bufs=1) as constp,
            tc.tile_pool(name="stp", bufs=4) as stp,
            tc.tile_pool(name="ypp", bufs=4) as ypp,
            tc.tile_pool(name="ztp", bufs=4) as ztp,
            tc.tile_pool(name="zpp", bufs=4) as zpp,
            tc.tile_pool(name="p2p", bufs=14) as p2p,
            tc.tile_pool(name="skp", bufs=3) as skp,
            tc.tile_pool(name="outp", bufs=2) as outp,
            tc.tile_pool(name="ps_yp", bufs=2, space="PSUM") as ps_yp,
            tc.tile_pool(name="ps_z", bufs=2, space="PSUM") as ps_z,
            tc.tile_pool(name="ps_zp", bufs=2, space="PSUM") as ps_zp,
            tc.tile_pool(name="ps_acc", bufs=2, space="PSUM") as ps_acc,
        ):
            w_sb = constp.tile([F, K * F], bf16)
            nc.sync.dma_start(out=w_sb[:], in_=w_d[:])
            bias_sb = constp.tile([1, F], bf16)
            nc.sync.dma_start(out=bias_sb[:], in_=bias_d[:])
            iota_sb = constp.tile([128, 128], bf16)
            nc.sync.dma_start(out=iota_sb[:], in_=iota_d[:])
            identity = constp.tile([128, 128], bf16)
            make_identity(nc, identity[:])
            ones_sb = constp.tile([1, 128], bf16)
            nc.vector.memset(ones_sb[:], 1.0)

            st_t = sk_t = None
            acc_t = None
            yp_ps = None
            p2_ring = {}

            for u in range(U):
                rnd, ur = divmod(u, RND)
                sux = ur % SU                    # unit index inside superunit

                if ur == 0:
                    st_t = stp.tile([128, RND * 641], bf16, tag="st")
                    lo = 0
                    for eng, frac in ST_SPLIT:
                        hi = (min(RND * 641, lo + int(RND * 641 * frac))
                              if frac else RND * 641)
                        getattr(nc, eng).dma_start(
                            out=st_t[:, lo:hi], in_=st_d[rnd][:, lo:hi])
                        lo = hi
                    # skey columns (bf16, strided) -> f32 for tensor_scalar
                    sk_t = skp.tile([128, RND], f32, tag="sk")
                    nc.vector.tensor_copy(
                        out=sk_t[:],
                        in_=st_t[:].rearrange("p (r c) -> p r c", c=641)
                        [:, :, 640:641].rearrange("p r c -> p (r c)"))

                if sux == 0:
                    yp_ps = ps_yp.tile([128, SU * 128], f32, tag="yp")

                # M1: Yp[fi, j] += G_cc^T @ P4w_cc  (4 matmuls, 32 cols each)
                for cc in range(4):
                    nc.tensor.matmul(
                        out=yp_ps[:, 128 * sux + 32 * cc:
                                  128 * sux + 32 * cc + 32],
                        lhsT=st_t[:, 641 * ur + F * cc:641 * ur + F * (cc + 1)],
                        rhs=st_t[:, 641 * ur + 512 + 32 * cc:
                                 641 * ur + 512 + 32 * cc + 32],
                        start=(sux == 0 and cc == 0),
                        stop=(sux == SU - 1 and cc == 3))

                # P2_u[j, d] = (iota_d == skey[j])   (DVE, per unit)
                p2 = p2p.tile([128, 128], bf16, tag="p2")
                nc.vector.tensor_scalar(
                    out=p2[:], in0=iota_sb[:],
                    scalar1=sk_t[:, ur:ur + 1],
                    scalar2=None, op0=mybir.AluOpType.is_equal)
                p2_ring[u] = p2

                if sux != SU - 1:
                    continue

                # --- superunit boundary ---
                yp_sb = ypp.tile([128, SU * 128], bf16, tag="ypsb")
                nc.gpsimd.tensor_copy(out=yp_sb[:], in_=yp_ps[:])

                z_ps = ps_z.tile([128, SU * 128], f32, tag="z")
                yp_r = yp_sb[:].rearrange("p (s c k g) -> p s c k g",
                                          s=SU, c=4, k=K)
                for s in range(SU):
                    for k in range(K):
                        nc.tensor.matmul(
                            out=z_ps[:, 128 * s + 32 * k:
                                     128 * s + 32 * k + 32],
                            lhsT=w_sb[:, F * k:F * (k + 1)],
                            rhs=yp_r[:, s, :, k, :],
                            start=(s == 0 and k == 0),
                            stop=(s == SU - 1 and k == 3))
                zt_sb = ztp.tile([128, SU * 128], bf16, tag="ztsb")
                if (u // (SU * RND)) % 1 == 0 and (ur // SU) % 2 == 0:
                    nc.scalar.copy(out=zt_sb[:], in_=z_ps[:])
                else:
                    c0 = ZT_SPLIT
                    nc.gpsimd.tensor_copy(out=zt_sb[:, :c0],
                                          in_=z_ps[:, :c0])
                    nc.vector.tensor_copy(out=zt_sb[:, c0:],
                                          in_=z_ps[:, c0:])

                if ur // SU == 0:
                    zp_ps = ps_zp.tile([128, RND * 128], bf16, tag="zp")
                off = (ur // SU) * SU * 128
                for s in range(SU):
                    nc.tensor.transpose(
                        out=zp_ps[:, off + 128 * s:off + 128 * s + 128],
                        in_=zt_sb[:, 128 * s:128 * s + 128],
                        identity=identity[:])
                if ur != RND - 1:
                    continue
                zp_sb = zpp.tile([128, RND * 128], bf16, tag="zpsb")
                nc.vector.tensor_copy(out=zp_sb[:], in_=zp_ps[:])

                # M3 for the units of this round (may span blocks)
                for s in range(RND):
                    uu = u - (RND - 1) + s
                    bb_ = int(blk_of_unit[uu])
                    bat_, bslot_ = divmod(bb_, BB)
                    if uu == int(ustart[bb_]):
                        if bslot_ == 0:
                            acc_t = ps_acc.tile([128, BB * 128], f32,
                                                tag="acc")
                        nc.tensor.matmul(
                            out=acc_t[:, 128 * bslot_:128 * bslot_ + 128],
                            lhsT=ones_sb[:], rhs=bias_sb[:],
                            start=True, stop=False)
                    p2_s = p2_ring.pop(uu)
                    last = (uu == int(ustart[bb_ + 1]) - 1)
                    nc.tensor.matmul(
                        out=acc_t[:, 128 * bslot_:128 * bslot_ + 128],
                        lhsT=p2_s[:],
                        rhs=zp_sb[:, 128 * s:128 * s + 128],
                        start=False, stop=last)
                    if last and (bslot_ == BB - 1 or bb_ == NBLK - 1):
                        nb = bslot_ + 1
                        out_sb = outp.tile([128, BB * 128], f32, tag="out")
                        nc.gpsimd.tensor_copy(
                            out=out_sb[:, :128 * nb],
                            in_=acc_t[:, :128 * nb])
                        r0 = bat_ * BB * 128
                        nfull = min(nb, (RPC - r0) // 128)
                        if nfull > 0:
                            ov = out_d[r0:r0 + 128 * nfull, :].rearrange(
                                "(blk d) f -> d blk f", d=128)
                            nc.sync.dma_start(
                                out=ov,
                                in_=out_sb[:, :128 * nfull].rearrange(
                                    "d (blk f) -> d blk f", f=F))
                        if nfull < nb:
                            rows = RPC - r0 - 128 * nfull
                            nc.sync.dma_start(
                                out=out_d[r0 + 128 * nfull:RPC, :],
                                in_=out_sb[:rows,
                                           128 * nfull:128 * (nfull + 1)])
    nc.compile()
    return nc


# ----------------------------------------------------------------------------
# Entry points.
# ----------------------------------------------------------------------------

def _prepare(x, weight, bias, adj_vals, edge_row, edge_col):
    U_bs, payloads, devpos = _preprocess(adj_vals, edge_row, edge_col)
    nc = _build(U_bs)
    in_maps = _host_streams(x, weight, bias, payloads)
    return nc, in_maps, devpos


def kernel(x, weight, bias, adj_vals, edge_row, edge_col):
    from concourse.bass_utils import run_bass_kernel_spmd
    nc, in_maps, devpos = _prepare(x, weight, bias, adj_vals, edge_row,
                                   edge_col)
    res = run_bass_kernel_spmd(nc, in_maps, core_ids=list(range(N_CORES)))
    out_dev = np.concatenate(
        [np.asarray(res.results[c]["out"]) for c in range(N_CORES)], axis=0)
    return out_dev[devpos].astype(np.float32)


# revision 14
# speedup vs baseline: 1.0231x; 1.0231x over previous
"""Trainium2 Bass kernel: Chebyshev graph convolution.

Computes  out = sum_k A_k @ (x @ W_k) + bias  where A_k are sparse COO
matrices sharing one sparsity pattern (edge_row/edge_col) with per-degree
values adj_vals[k].

Restructured as:
    G      = x[edge_col]                       (host gather, once per edge)
    Y_k    = segment_sum(adj_vals[k] * G)      (grouped one-hot matmuls on PE)
    out    = sum_k Y_k @ W_k + bias            (dense, fused into the scatter)

Sharding: destination-node range across 8 cores (6250 rows each), zero
cross-core communication.

Device pipeline per superunit (4 units; one unit = 512 sorted+padded edge
slots = 32 groups of 16, each group summing into one dest row;
j = 32*cc + 8*k + g):
    G_cc[e, fi]      = x[col[e], :]              (streamed dense DMA)
    P4w[e, j]        = pattern[e,g] * a_k[e]     (pre-baked on HOST, streamed)
    Yp[fi, j]       += G_cc^T @ P4w_cc           (M1: 16 matmuls, bf16 PSUM)
    yp_sb            = copy(Yp)                  (Pool engine)
    ZT[fo, j]        = W_k^T @ Yp[:, kslice]     (M2: 16 matmuls, bf16 PSUM)
    zT_sb            = copy(ZT)                  (ACT engine)
    Zp_u             = ZT_u^T                    (PE transpose, per unit)
    zp_sb            = copy(Zp)                  (DVE engine)
    P2_u[j, d]       = (iota_d == skey[j])       (DVE tensor_scalar, per unit)
    acc[d, fo]      += P2_u^T @ Zp_u             (M3, f32 PSUM, 4 blocks per
                                                  acc tile, bias-seeded)
    out[4 blocks]    = copy(acc) ; DMA out       (Pool copy, one DMA)
"""

import numpy as np

N_NODES = 50000
N_EDGES = 800000
F = 128
K = 4
N_CORES = 8
RPC = N_NODES // N_CORES      # rows per core
NBLK = (RPC + 127) // 128     # dest blocks per core (49)
GS = 16                       # edges per group (one dest row per group)
GPU = 32                      # groups per unit
EPU = GS * GPU                # edges per unit (512)
SU = 4                        # units per superunit (PSUM bank = [128, 512])
RND = 8                       # units per staging round (2 superunits)
BB = 4                        # blocks per output batch

F32 = np.float32
I32 = np.int32

USE_FP8_G = False             # fp8e4m3 for the gathered-feature stream


# ----------------------------------------------------------------------------
# Host-side preprocessing: shard + sort + pad the edge list, build payloads.
# ----------------------------------------------------------------------------

def _preprocess(adj_vals, edge_row, edge_col):
    """Build per-core payload arrays and the shared static schedule.

    Returns (U_bs, payloads): U_bs[b] = units of block b (shared by all
    cores; padded so sum(U_bs) % RND == 0); payloads[c] = dict(cols_unit
    [U*512] int32, a4 [U*512, K] f32, skey_j [U, 128] f32).
    """
    import heapq

    er = np.asarray(edge_row, dtype=np.int64)
    ec = np.asarray(edge_col, dtype=np.int64)
    adj = np.asarray(adj_vals, dtype=F32)            # [K, E]

    # Balanced dest-row assignment: LPT-pack rows into the 392 (core, block)
    # bins by group count, so every block needs the same number of units on
    # every core (U_bs is a max over cores).
    counts_all = np.bincount(er, minlength=N_NODES)
    g_all = -(-counts_all // GS)
    nbins = N_CORES * NBLK
    cap = np.full(nbins, 128, dtype=np.int64)
    cap[NBLK - 1::NBLK] = RPC - 128 * (NBLK - 1)
    fill = np.zeros(nbins, dtype=np.int64)
    assign_loc = np.empty(N_NODES, dtype=np.int64)   # core*RPC + local row
    heap = [(200 if i % NBLK == NBLK - 1 else 0, i) for i in range(nbins)]
    heapq.heapify(heap)
    for r in np.argsort(-g_all, kind="stable"):
        while True:
            load, b = heapq.heappop(heap)
            if fill[b] < cap[b]:
                break
        c_, blk = divmod(b, NBLK)
        assign_loc[r] = c_ * RPC + blk * 128 + fill[b]
        fill[b] += 1
        if fill[b] < cap[b]:
            heapq.heappush(heap, (load + int(g_all[r]), b))
    devpos_of_row = assign_loc

    er_loc = assign_loc[er]                          # device position per edge
    core = er_loc // RPC
    per_core = []
    gpb_all = np.zeros((N_CORES, NBLK), dtype=np.int64)
    for c in range(N_CORES):
        sel = np.nonzero(core == c)[0]
        rloc = (er_loc[sel] - c * RPC).astype(np.int64)
        order = np.argsort(rloc, kind="stable")
        eidx = sel[order]
        rs = rloc[order]
        counts = np.bincount(rs, minlength=RPC)       # edges per dest row
        gpr = -(-counts // GS)                        # groups per row
        gpr_pad = np.zeros(NBLK * 128, dtype=np.int64)
        gpr_pad[:RPC] = gpr
        gpb_all[c] = gpr_pad.reshape(NBLK, 128).sum(1)
        per_core.append((eidx, rs, counts, gpr))

    U_bs = np.maximum((-(-gpb_all // GPU)).max(axis=0), 1)  # units per block
    U = int(U_bs.sum())
    Upad = -(-U // RND) * RND
    U_bs[:Upad - U] += 1                   # spread pad units over first blocks
    U = Upad

    blk_grp_base = np.concatenate([[0], np.cumsum(U_bs * GPU)])[:-1]  # [NBLK]

    payloads = []
    for c in range(N_CORES):
        eidx, rs, counts, gpr = per_core[c]
        gpr_cum = np.concatenate([[0], np.cumsum(gpr)])  # [RPC+1]
        row_block = np.arange(RPC) // 128
        blk_first_row = row_block * 128
        grp_base_row = (blk_grp_base[row_block]
                        + gpr_cum[:RPC] - gpr_cum[blk_first_row])
        slot_base_row = grp_base_row * GS

        # scatter real edges into padded slots
        row_start = np.concatenate([[0], np.cumsum(counts)])  # [RPC+1]
        within = np.arange(len(rs)) - row_start[rs]
        pos = slot_base_row[rs] + within

        nslot = U * EPU
        cols_flat = np.zeros(nslot, dtype=I32)
        a4_flat = np.zeros((nslot, K), dtype=F32)
        cols_flat[pos] = ec[eidx].astype(I32)
        a4_flat[pos] = adj[:, eidx].T

        # dest row (mod 128) per group
        ngrp = U * GPU
        skey_grp = np.zeros(ngrp, dtype=F32)
        totg = int(gpr.sum())
        row_ids = np.repeat(np.arange(RPC), gpr)
        grp_within = np.arange(totg) - np.repeat(gpr_cum[:RPC], gpr)
        grp_pos = np.repeat(grp_base_row, gpr) + grp_within
        skey_grp[grp_pos] = (row_ids % 128).astype(F32)

        # per-unit skey over zp rows: j2 = 32*k + 8*cc + g  ->  group cc*8+g
        skey_j = np.tile(skey_grp.reshape(U, GPU), (1, K))

        payloads.append(dict(cols_unit=cols_flat, a4=a4_flat, skey_j=skey_j,
                             n_units=U))
    return U_bs.astype(np.int64), payloads, devpos_of_row


def _np_dt(dt_name):
    if dt_name == "bf16":
        import ml_dtypes
        return ml_dtypes.bfloat16
    if dt_name == "fp8":
        import ml_dtypes
        return ml_dtypes.float8_e4m3fn
    return F32


def _host_streams(x, weight, bias, payloads):
    """Per-core DMA streams, round-major so each DMA has one big contiguous
    descriptor per partition:
      g  [NRND, 128, RND*4*F]  (gdt)   unit ur chunk cc at col 512*ur+128*cc
      pw [NRND, 128, RND*129]  (bf16)  unit ur: 128 P4w cols + 1 skey col
    plus small consts."""
    bf = _np_dt("bf16")
    gdt = _np_dt("fp8" if USE_FP8_G else "bf16")
    xg = np.ascontiguousarray(np.asarray(x, dtype=F32).astype(gdt))
    w_packed = np.ascontiguousarray(
        np.asarray(weight, dtype=F32).transpose(1, 0, 2)
        .reshape(F, K * F).astype(bf))
    bias_row = np.asarray(bias, dtype=F32)[None, :].astype(bf).copy()
    iota = np.broadcast_to(
        np.arange(128, dtype=F32)[None, :], (128, 128)).astype(bf).copy()

    e_ar = np.arange(128)
    onehot_g = (e_ar[:, None] // GS == np.arange(8)[None, :]).astype(F32)

    in_maps = []
    for pl in payloads:
        U = pl["n_units"]
        NRND = U // RND
        st = np.empty((U, 128, 641), dtype=bf)
        # g block: [U, e, cc, fi]
        st[:, :, :512] = (xg[pl["cols_unit"]].astype(bf)
                          .reshape(U, 4, 128, F).transpose(0, 2, 1, 3)
                          .reshape(U, 128, 4 * F))
        # p4w: [U, e, 32cc+8k+g] = a4[slot(u,cc,e), k] * (e//16 == g)
        a4r = pl["a4"].reshape(U, 4, 128, K)             # [U, cc, e, k]
        st[:, :, 512:640] = np.einsum(
            "ucek,eg->ueckg", a4r, onehot_g).astype(bf).reshape(U, 128, 128)
        st[:, :, 640] = pl["skey_j"].astype(bf)
        st_rm = np.ascontiguousarray(
            st.reshape(NRND, RND, 128, 641).transpose(0, 2, 1, 3)
            .reshape(NRND, 128, RND * 641))
        in_maps.append({
            "st": st_rm,
            "w": w_packed, "bias_row": bias_row, "iota": iota,
        })
    return in_maps


# ----------------------------------------------------------------------------
# Numpy emulation of the device pipeline (layout validation / debugging).
# ----------------------------------------------------------------------------

def _emulate(x, weight, bias, U_bs, payloads):
    """Returns the DEVICE-ORDER output; index with devpos_of_row to compare."""
    in_maps = _host_streams(x, weight, bias, payloads)
    out = np.zeros((N_NODES, F), dtype=F32)
    blk_of_unit = np.repeat(np.arange(NBLK), U_bs)
    for c in range(N_CORES):
        im = in_maps[c]
        NRND = im["st"].shape[0]
        U = NRND * RND
        st = (im["st"].reshape(NRND, 128, RND, 641).transpose(0, 2, 1, 3)
              .reshape(U, 128, 641).astype(F32))
        g = st[:, :, :512].reshape(U, 128, 4, F)
        pw = st[:, :, 512:]
        w = im["w"].astype(F32)                       # [fi, K*F]
        iota = im["iota"].astype(F32)
        acc = np.zeros((NBLK, 128, F), dtype=F32)
        for u in range(U):
            b = int(blk_of_unit[u])
            yp = np.zeros((F, 128), dtype=F32)
            for cc in range(4):
                yp[:, 32 * cc:32 * cc + 32] = (
                    g[u, :, cc, :].T @ pw[u, :, 32 * cc:32 * cc + 32])
            zT = np.zeros((F, 128), dtype=F32)
            ypr = yp.reshape(F, 4, K, 8)
            for k in range(K):
                zT[:, 32 * k:32 * k + 32] = (
                    w[:, F * k:F * (k + 1)].T
                    @ ypr[:, :, k, :].reshape(F, 32))
            zp = zT.T
            p2 = (iota == pw[u, :, 128][:, None]).astype(F32)
            acc[b] += p2.T @ zp
        for b in range(NBLK):
            rows = min(128, RPC - 128 * b)
            out[c * RPC + 128 * b: c * RPC + 128 * b + rows] = (
                acc[b, :rows] + np.asarray(bias, F32)[None, :])
    return out


# ----------------------------------------------------------------------------
# Bass kernel builder.
# ----------------------------------------------------------------------------

def _build(U_bs):
    import concourse.bacc as bacc
    import concourse.mybir as mybir
    import concourse.tile as tile
    from concourse.masks import make_identity

    f32 = mybir.dt.float32
    bf16 = mybir.dt.bfloat16
    gdt = mybir.dt.float8e4 if USE_FP8_G else bf16
    # stream DMA split (engine, fraction of round); last gets remainder
    ST_SPLIT = (("sync", 0.44), ("scalar", 0.35), ("gpsimd", None))
    # odd-superunit zT copy: [0:c0] Pool, [c0:] DVE
    ZT_SPLIT = 256

    U = int(U_bs.sum())
    NRND = U // RND

    nc = bacc.Bacc("TRN2", target_bir_lowering=False, debug=False,
                   enable_asserts=False, num_devices=N_CORES)

    st_d = nc.dram_tensor("st", [NRND, 128, RND * 641], bf16,
                          kind="ExternalInput")
    w_d = nc.dram_tensor("w", [F, K * F], bf16, kind="ExternalInput")
    bias_d = nc.dram_tensor("bias_row", [1, F], bf16, kind="ExternalInput")
    iota_d = nc.dram_tensor("iota", [128, 128], bf16, kind="ExternalInput")
    out_d = nc.dram_tensor("out", [RPC, F], f32, kind="ExternalOutput")

    blk_of_unit = np.repeat(np.arange(NBLK), U_bs)
    ustart = np.concatenate([[0], np.cumsum(U_bs)])

    with tile.TileContext(nc) as tc:
        with (
            tc.tile_pool(name="const", bufs=1) as constp,
            tc.tile_pool(name="stp", bufs=4) as stp,
            tc.tile_pool(name="ypp", bufs=4) as ypp,
            tc.tile_pool(name="ztp", bufs=4) as ztp,
            tc.tile_pool(name="zpp", bufs=4) as zpp,
            tc.tile_pool(name="p2p", bufs=14) as p2p,
            tc.tile_pool(name="skp", bufs=3) as skp,
            tc.tile_pool(name="outp", bufs=2) as outp,
            tc.tile_pool(name="ps_yp", bufs=2, space="PSUM") as ps_yp,
            tc.tile_pool(name="ps_z", bufs=2, space="PSUM") as ps_z,
            tc.tile_pool(name="ps_zp", bufs=2, space="PSUM") as ps_zp,
            tc.tile_pool(name="ps_acc", bufs=2, space="PSUM") as ps_acc,
        ):
            w_sb = constp.tile([F, K * F], bf16)
            nc.sync.dma_start(out=w_sb[:], in_=w_d[:])
            bias_sb = constp.tile([1, F], bf16)
            nc.sync.dma_start(out=bias_sb[:], in_=bias_d[:])
            iota_sb = constp.tile([128, 128], bf16)
            nc.sync.dma_start(out=iota_sb[:], in_=iota_d[:])
            identity = constp.tile([128, 128], bf16)
            make_identity(nc, identity[:])
            ones_sb = constp.tile([1, 128], bf16)
            nc.vector.memset(ones_sb[:], 1.0)

            st_t = sk_t = None
            acc_t = None
            yp_ps = None
            p2_ring = {}

            for u in range(U):
                rnd, ur = divmod(u, RND)
                sux = ur % SU                    # unit index inside superunit

                if ur == 0:
                    st_t = stp.tile([128, RND * 641], bf16, tag="st")
                    lo = 0
                    for eng, frac in ST_SPLIT:
                        hi = (min(RND * 641, lo + int(RND * 641 * frac))
                              if frac else RND * 641)
                        getattr(nc, eng).dma_start(
                            out=st_t[:, lo:hi], in_=st_d[rnd][:, lo:hi])
                        lo = hi
                    # skey columns (bf16, strided) -> f32 for tensor_scalar
                    sk_t = skp.tile([128, RND], f32, tag="sk")
                    nc.vector.tensor_copy(
                        out=sk_t[:],
                        in_=st_t[:].rearrange("p (r c) -> p r c", c=641)
                        [:, :, 640:641].rearrange("p r c -> p (r c)"))

                if sux == 0:
                    yp_ps = ps_yp.tile([128, SU * 128], f32, tag="yp")

                # M1: Yp[fi, j] += G_cc^T @ P4w_cc  (4 matmuls, 32 cols each)
                for cc in range(4):
                    nc.tensor.matmul(
                        out=yp_ps[:, 128 * sux + 32 * cc:
                                  128 * sux + 32 * cc + 32],
                        lhsT=st_t[:, 641 * ur + F * cc:641 * ur + F * (cc + 1)],
                        rhs=st_t[:, 641 * ur + 512 + 32 * cc:
                                 641 * ur + 512 + 32 * cc + 32],
                        start=(sux == 0 and cc == 0),
                        stop=(sux == SU - 1 and cc == 3))

                # P2_u[j, d] = (iota_d == skey[j])   (DVE, per unit)
                p2 = p2p.tile([128, 128], bf16, tag="p2")
                nc.vector.tensor_scalar(
                    out=p2[:], in0=iota_sb[:],
                    scalar1=sk_t[:, ur:ur + 1],
                    scalar2=None, op0=mybir.AluOpType.is_equal)
                p2_ring[u] = p2

                if sux != SU - 1:
                    continue

                # --- superunit boundary ---
                yp_sb = ypp.tile([128, SU * 128], bf16, tag="ypsb")
                nc.gpsimd.tensor_copy(out=yp_sb[:], in_=yp_ps[:])

                z_ps = ps_z.tile([128, SU * 128], f32, tag="z")
                yp_r = yp_sb[:].rearrange("p (s c k g) -> p s c k g",
                                          s=SU, c=4, k=K)
                for s in range(SU):
                    for k in range(K):
                        nc.tensor.matmul(
                            out=z_ps[:, 128 * s + 32 * k:
                                     128 * s + 32 * k + 32],
                            lhsT=w_sb[:, F * k:F * (k + 1)],
                            rhs=yp_r[:, s, :, k, :],
                            start=(s == 0 and k == 0),
                            stop=(s == SU - 1 and k == 3))
                zt_sb = ztp.tile([128, SU * 128], bf16, tag="ztsb")
                if (u // (SU * RND)) % 1 == 0 and (ur // SU) % 2 == 0:
                    nc.scalar.copy(out=zt_sb[:], in_=z_ps[:])
                else:
                    c0 = ZT_SPLIT
                    nc.gpsimd.tensor_copy(out=zt_sb[:, :c0],
                                          in_=z_ps[:, :c0])
                    nc.vector.tensor_copy(out=zt_sb[:, c0:],
                                          in_=z_ps[:, c0:])

                if ur // SU == 0:
                    zp_ps = ps_zp.tile([128, RND * 128], bf16, tag="zp")
                off = (ur // SU) * SU * 128
                for s in range(SU):
                    nc.tensor.transpose(
                        out=zp_ps[:, off + 128 * s:off + 128 * s + 128],
                        in_=zt_sb[:, 128 * s:128 * s + 128],
                        identity=identity[:])
                if ur != RND - 1:
                    continue
                zp_sb = zpp.tile([128, RND * 128], bf16, tag="zpsb")
                nc.vector.tensor_copy(out=zp_sb[:], in_=zp_ps[:])

                # M3 for the units of this round (may span blocks)
                for s in range(RND):
                    uu = u - (RND - 1) + s
                    bb_ = int(blk_of_unit[uu])
                    bat_, bslot_ = divmod(bb_, BB)
                    if uu == int(ustart[bb_]):
                        if bslot_ == 0:
                            acc_t = ps_acc.tile([128, BB * 128], f32,
                                                tag="acc")
                        nc.tensor.matmul(
                            out=acc_t[:, 128 * bslot_:128 * bslot_ + 128],
                            lhsT=ones_sb[:], rhs=bias_sb[:],
                            start=True, stop=False)
                    p2_s = p2_ring.pop(uu)
                    last = (uu == int(ustart[bb_ + 1]) - 1)
                    nc.tensor.matmul(
                        out=acc_t[:, 128 * bslot_:128 * bslot_ + 128],
                        lhsT=p2_s[:],
                        rhs=zp_sb[:, 128 * s:128 * s + 128],
                        start=False, stop=last)
                    if last and (bslot_ == BB - 1 or bb_ == NBLK - 1):
                        nb = bslot_ + 1
                        out_sb = outp.tile([128, BB * 128], f32, tag="out")
                        nc.gpsimd.tensor_copy(
                            out=out_sb[:, :128 * nb],
                            in_=acc_t[:, :128 * nb])
                        r0 = bat_ * BB * 128
                        nfull = min(nb, (RPC - r0) // 128)
                        if nfull > 0:
                            ov = out_d[r0:r0 + 128 * nfull, :].rearrange(
                                "(blk d) f -> d blk f", d=128)
                            nc.sync.dma_start(
                                out=ov,
                                in_=out_sb[:, :128 * nfull].rearrange(
                                    "d (blk f) -> d blk f", f=F))
                        if nfull < nb:
                            rows = RPC - r0 - 128 * nfull
                            nc.sync.dma_start(
                                out=out_d[r0 + 128 * nfull:RPC, :],
                                in_=out_sb[:rows,
                                           128 * nfull:128 * (nfull + 1)])
    nc.compile()
    return nc


# ----------------------------------------------------------------------------
# Entry points.
# ----------------------------------------------------------------------------

def _prepare(x, weight, bias, adj_vals, edge_row, edge_col):
    U_bs, payloads, devpos = _preprocess(adj_vals, edge_row, edge_col)
    nc = _build(U_bs)
    in_maps = _host_streams(x, weight, bias, payloads)
    return nc, in_maps, devpos


def kernel(x, weight, bias, adj_vals, edge_row, edge_col):
    from concourse.bass_utils import run_bass_kernel_spmd
    nc, in_maps, devpos = _prepare(x, weight, bias, adj_vals, edge_row,
                                   edge_col)
    res = run_bass_kernel_spmd(nc, in_maps, core_ids=list(range(N_CORES)))
    out_dev = np.concatenate(
        [np.asarray(res.results[c]["out"]) for c in range(N_CORES)], axis=0)
    return out_dev[devpos].astype(np.float32)
